# revision 2
# baseline (speedup 1.0000x reference)
"""Trainium2 Bass kernel for nn_DenoiseNet — fp8 DoubleRow redesign.

Strategy (8 NeuronCores, SPMD, zero inter-core communication):
  - Shard over B(2) x 4 contiguous N-chunks of 4096 points with 64-point
    halo (dependency cone), global-edge clipping via per-core weight data.
  - Residual adds ride the TensorEngine: layer2 computes
    Wb2·(h0+r1) as one fp8 DoubleRow matmul pairing [Wb2;Wb2]x[h0;r1]
    (0.5 cycles/col), so no elementwise adds exist at all. The only
    elementwise work is one PSUM->SBUF relu/convert crossing per layer,
    k-batched into [128, 4x512] blocks split across ACT and DVE.
  - Activations h0/r1/r2 stored fp8(e4m3) in an 8-way interleaved layout
    col(p,k,m) = 8(m+1)+4p+k so that layer and scatter DoubleRow pairs
    (h0_k,r1_k) and shifted scatter reads are all lattice access patterns.
  - Scatter k-sum accumulates in PSUM; langevin update delta_out =
    s_i*psum + delta_in fused into one scalar_tensor_tensor crossing.
"""

import sys
import numpy as np

for _p in ("/opt/trn_rl_repo",):
    if _p not in sys.path:
        sys.path.insert(0, _p)

import concourse.bass as bass
import concourse.bacc as bacc
import concourse.tile as tile
from concourse import mybir
from concourse.bass_utils import run_bass_kernel_spmd

# ---- problem constants (hardcoded per harness contract) ----
B, N, D = 2, 16384, 3
F = 128
K = 4
OFF = [-2, -1, 0, 1]
STEPS, S0, DECAY = 4, 0.2, 0.95
CHUNK, HALO, GW = 4096, 64, 2
NP = CHUNK + 2 * HALO          # 4224 local points
NB = NP + 2 * GW               # 4228 delta/pcl cols (with guards)
N_CORES = 8

MT = NP + 4                    # m-tilde count per region (spill margins)
HRSZ = 8 * MT                  # h0/r1 interleaved region bytes/partition
R2SZ = 4 * MT                  # r2 region
HHSZ = HRSZ + R2SZ

f32 = mybir.dt.float32
f16 = mybir.dt.float16
f8 = mybir.dt.float8e4
AF = mybir.ActivationFunctionType
ALU = mybir.AluOpType
PM = mybir.MatmulPerfMode

# 512-col blocks over NP (last is 128)
_BLK = [(c * 512, min(512, NP - c * 512)) for c in range((NP + 511) // 512)]
# 2048-col scatter tiles over NP (last is 128)
_SCT = [(t * 2048, min(2048, NP - t * 2048)) for t in range((NP + 2047) // 2048)]
SVALS = [S0 * DECAY ** i for i in range(STEPS)]


def build_program(reps=1, loop_n=0):
    nc = bacc.Bacc("TRN2", target_bir_lowering=False, debug=False)

    def inp(name, shape, dt):
        return nc.dram_tensor(name, list(shape), dt, kind="ExternalInput").ap()

    d_pclT = inp("pclT", (4, NB), f16)
    d_delta0 = inp("delta0", (4, NB), f16)
    d_Wf1 = inp("Wf1", (3, F), f16)
    d_bf1 = inp("bf1", (F, 1), f32)
    d_WfW = inp("WfW", (F, F), f16)
    d_bg = inp("bg", (F, 1), f32)
    d_W0g = inp("W0g", (3, F), f16)
    d_W0gn = inp("W0gn", (3, F), f16)
    d_I128p = inp("I128p", (F, 2, F), f8)
    d_Wb1d = inp("Wb1d", (F, 2, F), f8)
    d_Wb2d = inp("Wb2d", (F, 2, F), f8)
    d_bb1 = inp("bb1", (F, 1), f32)
    d_bb2 = inp("bb2", (F, 1), f32)
    d_Wod = inp("Wod", (F, 2, 16), f8)
    d_Wos = inp("Wos", (F, 2, 16), f8)
    d_bo1 = inp("bo1", (4, 3), f16)
    d_eLp = inp("eLp", (F, 2, 16), f8)
    d_eLs = inp("eLs", (F, 2, 16), f8)
    d_eLnp = inp("eLnp", (F, 2, 16), f8)
    d_eLns = inp("eLns", (F, 2, 16), f8)
    d_eRp = inp("eRp", (F, 2, 16), f8)
    d_eRs = inp("eRs", (F, 2, 16), f8)
    d_eRnp = inp("eRnp", (F, 2, 16), f8)
    d_eRns = inp("eRns", (F, 2, 16), f8)
    d_flagL = inp("flagL", (4, 1), f32)
    d_flagR = inp("flagR", (4, 1), f32)
    d_out = nc.dram_tensor("outT", [4, CHUNK], f16, kind="ExternalOutput").ap()

    from contextlib import ExitStack
    with tile.TileContext(nc) as tc, ExitStack() as ctx:
        cpool = ctx.enter_context(tc.tile_pool(name="const", bufs=1))
        hfpool = ctx.enter_context(tc.tile_pool(name="hf", bufs=2))
        tpool = ctx.enter_context(tc.tile_pool(name="tiny", bufs=2))
        stpool = ctx.enter_context(tc.tile_pool(name="scst", bufs=3))
        psp = ctx.enter_context(tc.tile_pool(name="ps", bufs=4, space="PSUM"))

        def load(dram, shape, dt, tag):
            t = cpool.tile(list(shape), dt, tag=tag, name=tag)
            nc.sync.dma_start(t[:], dram[:])
            return t

        pclT = load(d_pclT, (4, NB), f16, "pclT")
        delta_a = load(d_delta0, (4, NB), f16, "delta_a")
        delta_b = load(d_delta0, (4, NB), f16, "delta_b")
        Wf1 = load(d_Wf1, (3, F), f16, "Wf1")
        bf1 = load(d_bf1, (F, 1), f32, "bf1")
        WfW = load(d_WfW, (F, F), f16, "WfW")
        bg = load(d_bg, (F, 1), f32, "bg")
        W0g = load(d_W0g, (3, F), f16, "W0g")
        W0gn = load(d_W0gn, (3, F), f16, "W0gn")
        I128p = load(d_I128p, (F, 2, F), f8, "I128p")
        Wb1d = load(d_Wb1d, (F, 2, F), f8, "Wb1d")
        Wb2d = load(d_Wb2d, (F, 2, F), f8, "Wb2d")
        bb1 = load(d_bb1, (F, 1), f32, "bb1")
        bb2 = load(d_bb2, (F, 1), f32, "bb2")
        Wod = load(d_Wod, (F, 2, 16), f8, "Wod")
        Wos = load(d_Wos, (F, 2, 16), f8, "Wos")
        bo1 = load(d_bo1, (4, 3), f16, "bo1")
        eLp = load(d_eLp, (F, 2, 16), f8, "eLp")
        eLs = load(d_eLs, (F, 2, 16), f8, "eLs")
        eLnp = load(d_eLnp, (F, 2, 16), f8, "eLnp")
        eLns = load(d_eLns, (F, 2, 16), f8, "eLns")
        eRp = load(d_eRp, (F, 2, 16), f8, "eRp")
        eRs = load(d_eRs, (F, 2, 16), f8, "eRs")
        eRnp = load(d_eRnp, (F, 2, 16), f8, "eRnp")
        eRns = load(d_eRns, (F, 2, 16), f8, "eRns")
        flagL = load(d_flagL, (4, 1), f32, "flagL")
        flagR = load(d_flagR, (4, 1), f32, "flagR")

        # persistent activation store: h0/r1 interleaved region + r2 region
        HH = cpool.tile([F, HHSZ], f8, tag="HH", name="HH")
        # views: hr8[e, mt] at byte 8*mt+e ; r24[e, mt] at HRSZ + 4*mt+e
        hr8 = HH[:, 0:HRSZ].rearrange("p (m e) -> p e m", e=8)
        r24 = HH[:, HRSZ:HHSZ].rearrange("p (m e) -> p e m", e=4)
        # phase views of the r2 region for k-paired scatter reads
        r2ph1 = HH[:, HRSZ + 1: HRSZ + 1 + 4 * (NP + 3)].rearrange(
            "p (m e) -> p e m", e=4)
        r2ph3 = HH[:, HRSZ + 3: HRSZ + 3 + 4 * (NP + 3)].rearrange(
            "p (m e) -> p e m", e=4)
        Gk = cpool.tile([F, K, NP], f8, tag="Gk", name="Gk")
        A0e = cpool.tile([F, NB], f16, tag="A0e", name="A0e")
        G0 = cpool.tile([F, NP], f16, tag="G0", name="G0")

        # spill-margin memsets (scatter shift reads touch these)
        nc.vector.memset(HH[:, 0:8], 0.0)
        nc.vector.memset(HH[:, 8 * (NP + 2):HRSZ], 0.0)
        nc.vector.memset(HH[:, HRSZ:HRSZ + 8], 0.0)
        nc.vector.memset(HH[:, HRSZ + 4 * (NP + 3):HHSZ], 0.0)

        # ---- greedy ACT/DVE balancer for PSUM->SBUF crossings ----
        eng_ns = {"ACT": 0.0, "DVE": 0.0}

        def relu_cross(dst, src, cols, bias=None):
            ca = (cols + 222) * 0.833
            cd = (cols + 120) * 1.042
            if eng_ns["ACT"] + ca <= eng_ns["DVE"] + cd:
                eng_ns["ACT"] += ca
                nc.scalar.activation(dst, src, AF.Relu,
                                     bias=(bias[:, :] if bias is not None else 0.0))
            else:
                eng_ns["DVE"] += cd
                if bias is not None:
                    nc.vector.tensor_scalar(dst, src, bias[:, :], 0.0, ALU.add, ALU.max)
                else:
                    nc.vector.tensor_scalar_max(dst, src, 0.0)

        def copy_cross(dst, src, cols, bias=None):
            ca = (cols + 222) * 0.833
            cd = (cols + 120) * 1.042
            if eng_ns["ACT"] + ca <= eng_ns["DVE"] + cd:
                eng_ns["ACT"] += ca
                if bias is not None:
                    nc.scalar.activation(dst, src, AF.Identity, bias=bias[:, :])
                else:
                    nc.scalar.activation(dst, src, AF.Copy)
            else:
                eng_ns["DVE"] += cd
                if bias is not None:
                    nc.vector.tensor_scalar_add(dst, src, bias[:, :])
                else:
                    nc.vector.tensor_copy(dst, src)

        # ---------------- preamble: A0e, G0, Gk ----------------
        nbt = [(t * 1024, min(1024, NB - t * 1024)) for t in range((NB + 1023) // 1024)]
        for c0, ext in nbt:
            ps = psp.tile([F, 2, 512], f32, tag="blk", name="ps_a0e")
            nbk = (ext + 511) // 512
            for b in range(nbk):
                fd = min(512, ext - b * 512)
                nc.tensor.matmul(ps[:, b, :fd], W0g[:, :],
                                 pclT[0:3, c0 + b * 512: c0 + b * 512 + fd],
                                 start=True, stop=True)
            if ext == 1024:
                copy_cross(A0e[:, c0:c0 + ext].rearrange("p (b m) -> p b m", b=2),
                           ps[:, :, :], ext)
            else:
                for b in range(nbk):
                    fd = min(512, ext - b * 512)
                    copy_cross(A0e[:, c0 + b * 512: c0 + b * 512 + fd],
                               ps[:, b, :fd], fd)

        npt = [(t * 1024, min(1024, NP - t * 1024)) for t in range((NP + 1023) // 1024)]
        for c0, ext in npt:
            ps = psp.tile([F, 2, 512], f32, tag="blk", name="ps_g0a")
            nbk = (ext + 511) // 512
            for b in range(nbk):
                fd = min(512, ext - b * 512)
                nc.tensor.matmul(ps[:, b, :fd], Wf1[:, :],
                                 pclT[0:3, GW + c0 + b * 512: GW + c0 + b * 512 + fd],
                                 start=True, stop=True)
            hf = hfpool.tile([F, 2, 512], f16, tag="hf", name="hf")
            if ext == 1024:
                relu_cross(hf[:, :, :], ps[:, :, :], ext, bias=bf1)
            else:
                relu_cross(hf[:, 0, :ext], ps[:, 0, :ext], ext, bias=bf1)
            ps2 = psp.tile([F, 2, 512], f32, tag="blk", name="ps_g0b")
            for b in range(nbk):
                fd = min(512, ext - b * 512)
                nc.tensor.matmul(ps2[:, b, :fd], WfW[:, :], hf[:, b, :fd],
                                 start=True, stop=False)
                nc.tensor.matmul(ps2[:, b, :fd], W0gn[:, :],
                                 pclT[0:3, GW + c0 + b * 512: GW + c0 + b * 512 + fd],
                                 start=False, stop=True)
            if ext == 1024:
                copy_cross(G0[:, c0:c0 + ext].rearrange("p (b m) -> p b m", b=2),
                           ps2[:, :, :], ext, bias=bg)
            else:
                copy_cross(G0[:, c0:c0 + ext], ps2[:, 0, :ext], ext, bias=bg)

        # Gk[k, n] = G0[n] + A0e[GW+off_k+n]   (fp16, 2x DVE / gpsimd mix)
        for k in range(K):
            for c0, fd in _BLK:
                nc.vector.tensor_add(Gk[:, k, c0:c0 + fd], G0[:, c0:c0 + fd],
                                     A0e[:, GW + OFF[k] + c0: GW + OFF[k] + c0 + fd])

        # ---------------- langevin steps ----------------
        NBLK = len(_BLK)                  # 9 n-blocks of <=512
        NSC = (NP + 1023) // 1024         # 5 scatter tiles of <=1024

        def emit_rep(final_rep):
            for step in range(STEPS):
                d_in = delta_a if step % 2 == 0 else delta_b
                d_out_t = delta_b if step % 2 == 0 else delta_a
                final = (step == STEPS - 1) and final_rep
                sv = SVALS[step]

                def emit_L0(bi, kh):
                    c0, fd = _BLK[bi]
                    ps = psp.tile([F, 2, 512], f32, tag="blk", name="ps_l0")
                    for j in range(2):
                        k = 2 * kh + j
                        nc.tensor.matmul(
                            ps[:, j, :fd], W0g[:, :],
                            d_in[0:3, GW + OFF[k] + c0: GW + OFF[k] + c0 + fd],
                            start=True, stop=False)
                        nc.tensor.matmul(ps[:, j, :fd], I128p[:, :, :],
                                         Gk[:, k:k + 1, c0:c0 + fd]
                                         .broadcast_to([F, 2, fd]),
                                         start=False, stop=True,
                                         perf_mode=PM.DoubleRow)
                    relu_cross(hr8[:, 2 * kh: 2 * kh + 2, c0 + 1: c0 + 1 + fd],
                               ps[:, :, :fd], 2 * fd)

                def emit_L1(bi, kh):
                    c0, fd = _BLK[bi]
                    ps = psp.tile([F, 2, 512], f32, tag="blk", name="ps_l1")
                    for j in range(2):
                        k = 2 * kh + j
                        rhs = hr8[:, k:k + 1, c0 + 1: c0 + 1 + fd].broadcast_to([F, 2, fd])
                        nc.tensor.matmul(ps[:, j, :fd], Wb1d[:, :, :], rhs,
                                         start=True, stop=True, perf_mode=PM.DoubleRow)
                    relu_cross(hr8[:, 4 + 2 * kh: 6 + 2 * kh, c0 + 1: c0 + 1 + fd],
                               ps[:, :, :fd], 2 * fd, bias=bb1)

                def emit_L2(bi, kh):
                    c0, fd = _BLK[bi]
                    ps = psp.tile([F, 2, 512], f32, tag="blk", name="ps_l2")
                    for j in range(2):
                        k = 2 * kh + j
                        rhs = hr8[:, k:8:4, c0 + 1: c0 + 1 + fd]
                        nc.tensor.matmul(ps[:, j, :fd], Wb2d[:, :, :], rhs,
                                         start=True, stop=True, perf_mode=PM.DoubleRow)
                    relu_cross(r24[:, 2 * kh: 2 * kh + 2, c0 + 2: c0 + 2 + fd],
                               ps[:, :, :fd], 2 * fd, bias=bb2)

                def emit_SC(ti):
                    t0 = ti * 1024
                    ext = min(1024, NP - t0)
                    ps = psp.tile([16, 2, 512], f32, tag="blk", name="ps_sc")
                    nbk = (ext + 511) // 512
                    for b in range(nbk):
                        n0 = t0 + b * 512
                        fd = min(512, ext - b * 512)
                        for k in range(K):
                            nc.tensor.matmul(
                                ps[0:16, b, :fd], Wod[:, :, :],
                                hr8[:, k:8:4, n0 + 3 - k: n0 + 3 - k + fd],
                                start=(k == 0), stop=False, perf_mode=PM.DoubleRow)
                        # r2 contributions: self-paired DR per k
                        for k in range(K):
                            nc.tensor.matmul(
                                ps[0:16, b, :fd], Wos[:, :, :],
                                r24[:, k:k + 1, n0 + 4 - k: n0 + 4 - k + fd]
                                .broadcast_to([F, 2, fd]),
                                start=False, stop=False, perf_mode=PM.DoubleRow)
                        # bank 0 of the first and last tiles receives edge
                        # matmuls afterwards; they carry the group stop there.
                        edge_bank = (ti == 0 or ti == NSC - 1) and b == 0
                        nc.tensor.matmul(ps[0:3, b, :fd], bo1[:, :],
                                         d_in[0:4, GW + n0: GW + n0 + fd],
                                         start=False, stop=not edge_bank)
                    if ti == 0:
                        # L-edge corrections at out col HALO (64, bank 0)
                        pcol = ps[0:16, 0, HALO:HALO + 1]
                        for (wp, ws, m) in ((eLp, eLs, HALO), (eLp, eLs, HALO + 1)):
                            nc.tensor.matmul(pcol, wp[:, :, :],
                                             hr8[:, 0:8:4, m + 1:m + 2],
                                             start=False, stop=False,
                                             perf_mode=PM.DoubleRow)
                            nc.tensor.matmul(pcol, ws[:, :, :],
                                             r24[:, 0:1, m + 2:m + 3]
                                             .broadcast_to([F, 2, 1]),
                                             start=False, stop=False,
                                             perf_mode=PM.DoubleRow)
                        # +(k1, m=HALO)
                        nc.tensor.matmul(pcol, eLp[:, :, :],
                                         hr8[:, 1:8:4, HALO + 1:HALO + 2],
                                         start=False, stop=False, perf_mode=PM.DoubleRow)
                        nc.tensor.matmul(pcol, eLs[:, :, :],
                                         r24[:, 1:2, HALO + 1:HALO + 2]
                                         .broadcast_to([F, 2, 1]),
                                         start=False, stop=False, perf_mode=PM.DoubleRow)
                        # -(k3, m=HALO-1)
                        nc.tensor.matmul(pcol, eLnp[:, :, :],
                                         hr8[:, 3:8:4, HALO:HALO + 1],
                                         start=False, stop=False, perf_mode=PM.DoubleRow)
                        nc.tensor.matmul(pcol, eLns[:, :, :],
                                         r24[:, 3:4, HALO + 1:HALO + 2]
                                         .broadcast_to([F, 2, 1]),
                                         start=False, stop=True, perf_mode=PM.DoubleRow)
                    if ti == NSC - 1:
                        lN = HALO + CHUNK - 1            # 4159, local col 63 in tile 4
                        lc = lN - t0
                        bq, cq = lc // 512, lc % 512
                        pN = ps[0:16, bq, cq:cq + 1]
                        # +(k3, m=lN)
                        nc.tensor.matmul(pN, eRp[:, :, :], hr8[:, 3:8:4, lN + 1:lN + 2],
                                         start=False, stop=False, perf_mode=PM.DoubleRow)
                        nc.tensor.matmul(pN, eRs[:, :, :],
                                         r24[:, 3:4, lN + 1:lN + 2]
                                         .broadcast_to([F, 2, 1]),
                                         start=False, stop=False, perf_mode=PM.DoubleRow)
                        # -(k0, m=lN+2), -(k1, m=lN+1)
                        for (kk, m) in ((0, lN + 2), (1, lN + 1)):
                            nc.tensor.matmul(pN, eRnp[:, :, :],
                                             hr8[:, kk:8:4, m + 1:m + 2],
                                             start=False, stop=False,
                                             perf_mode=PM.DoubleRow)
                            nc.tensor.matmul(pN, eRns[:, :, :],
                                             r24[:, kk:kk + 1, m + 2:m + 3]
                                             .broadcast_to([F, 2, 1]),
                                             start=False, stop=False,
                                             perf_mode=PM.DoubleRow)
                        # out col lN-1: -(k0, m=lN+1)
                        pM = ps[0:16, bq, cq - 1:cq]
                        nc.tensor.matmul(pM, eRnp[:, :, :],
                                         hr8[:, 0:8:4, lN + 2:lN + 3],
                                         start=False, stop=False, perf_mode=PM.DoubleRow)
                        nc.tensor.matmul(pM, eRns[:, :, :],
                                         r24[:, 0:1, lN + 3:lN + 4]
                                         .broadcast_to([F, 2, 1]),
                                         start=False, stop=True, perf_mode=PM.DoubleRow)
                    # crossing: delta_out = sv*psum + delta_in. Either fused
                    # on DVE (stt) or ACT scaled-copy + gpsimd add (frees DVE).
                    dst = d_out_t[0:3, GW + t0: GW + t0 + ext]
                    src_d = d_in[0:3, GW + t0: GW + t0 + ext]
                    pss = ps[0:3, :, :] if ext == 1024 else ps[0:3, 0, :ext]
                    if ext == 1024:
                        dst = dst.rearrange("p (b m) -> p b m", b=2)
                        src_d = src_d.rearrange("p (b m) -> p b m", b=2)
                    ca = (ext + 222) * 0.833
                    cd = (ext + 120) * 1.042
                    if eng_ns["DVE"] + cd <= eng_ns["ACT"] + ca:
                        eng_ns["DVE"] += cd
                        nc.vector.scalar_tensor_tensor(
                            dst, pss, float(sv), src_d, ALU.mult, ALU.add)
                    else:
                        eng_ns["ACT"] += ca
                        stage = stpool.tile([4, 2, 512], f16, tag="scst", name="scst")
                        stg = stage[0:3, :, :] if ext == 1024 else stage[0:3, 0, :ext]
                        nc.scalar.activation(stg, pss, AF.Copy, scale=float(sv))
                        nc.gpsimd.tensor_add(dst, stg, src_d)

                # mirror guards at global edges (gpsimd; no-op on interior)
                def mirror_fix(flag, src_l, dst_ls):
                    for dst_l in dst_ls:
                        t = tpool.tile([4, 1], f16, tag="mir", name="mir")
                        nc.gpsimd.tensor_sub(
                            t[0:3, :],
                            d_out_t[0:3, GW + src_l:GW + src_l + 1],
                            d_out_t[0:3, GW + dst_l:GW + dst_l + 1])
                        nc.gpsimd.tensor_scalar_mul(t[0:3, :], t[0:3, :],
                                                    flag[0:3, :])
                        nc.gpsimd.tensor_add(
                            d_out_t[0:3, GW + dst_l:GW + dst_l + 1],
                            d_out_t[0:3, GW + dst_l:GW + dst_l + 1],
                            t[0:3, :])

                # ---- interleaved pipeline over n-blocks ----
                for bi in range(NBLK + 6):
                    if bi < NBLK:
                        emit_L0(bi, 0)
                        emit_L0(bi, 1)
                    if 0 <= bi - 2 < NBLK:
                        emit_L1(bi - 2, 0)
                        emit_L1(bi - 2, 1)
                    if 0 <= bi - 4 < NBLK:
                        emit_L2(bi - 4, 0)
                        emit_L2(bi - 4, 1)
                    if bi >= 6 and (bi - 6) % 2 == 0 and (bi - 6) // 2 < NSC:
                        ti = (bi - 6) // 2
                        emit_SC(ti)
                        if not final and ti == 0:
                            mirror_fix(flagL, HALO, (HALO - 2, HALO - 1))
                        if not final and ti == NSC - 1:
                            mirror_fix(flagR, HALO + CHUNK - 1, (HALO + CHUNK,))

                if final:
                    nc.sync.dma_start(
                        d_out[:, :], d_out_t[0:4, GW + HALO:GW + HALO + CHUNK])

        if loop_n:
            with tc.For_i(0, loop_n, 1):
                emit_rep(False)
            emit_rep(True)
        else:
            for rep in range(reps):
                emit_rep(rep == reps - 1)

    nc.compile()
    return nc


def host_prep(inputs):
    """Slice/transpose/pad inputs per core; build fp8 weight pairs."""
    np8 = mybir.dt.np(f8)
    pcl = np.asarray(inputs["pcl_noisy"], np.float32)
    Wf1 = np.asarray(inputs["Wf1"], np.float32)
    bf1 = np.asarray(inputs["bf1"], np.float32)
    Wf2 = np.asarray(inputs["Wf2"], np.float32)
    bf2 = np.asarray(inputs["bf2"], np.float32)
    W0 = np.asarray(inputs["W0"], np.float32)
    b0 = np.asarray(inputs["b0"], np.float32)
    Wb = np.asarray(inputs["Wb"], np.float32)
    bb = np.asarray(inputs["bb"], np.float32)
    Wo = np.asarray(inputs["Wo"], np.float32)
    bo = np.asarray(inputs["bo"], np.float32)

    W0g = W0[:3]
    WfW = Wf2 @ W0[3:]
    bg = bf2 @ W0[3:] + b0
    offs = np.arange(-(K - 1) // 2, (K - 1) // 2 + 1)
    nbr = np.clip(np.arange(N)[:, None] + offs, 0, N - 1).reshape(-1)
    c_global = np.bincount(nbr, minlength=N).astype(np.float32)

    hf = np.float16
    zpair = np.zeros((F, F), np.float32)
    Wo8 = Wo.astype(np8)
    z3 = np.zeros((F, 3), np8)

    def pair(a, b):
        return np.stack([np.asarray(a, np.float32).astype(np8),
                         np.asarray(b, np.float32).astype(np8)], axis=1)

    shared = {
        "Wf1": Wf1.astype(hf), "bf1": bf1.reshape(F, 1),
        "WfW": WfW.astype(hf), "bg": bg.reshape(F, 1),
        "W0g": W0g.astype(hf), "W0gn": (-W0g).astype(hf),
        "I128p": pair(np.eye(F, dtype=np.float32), zpair),
        "Wb1d": pair(Wb[0], zpair),
        "Wb2d": pair(Wb[1], Wb[1]),
        "bb1": bb[0].reshape(F, 1), "bb2": bb[1].reshape(F, 1),
        "Wod": pair(Wo, Wo), "Wos": pair(Wo, np.zeros_like(Wo)),
        "bo1": np.concatenate([np.zeros((3, 3), np.float32),
                               bo.reshape(1, 3)], axis=0).astype(hf),
    }
    zpair3 = np.zeros((F, 2, 3), np8)
    in_maps = []
    for core in range(N_CORES):
        b, ch = core // 4, core % 4
        g0 = ch * CHUNK - HALO
        idx = np.clip(np.arange(g0 - GW, g0 + NP + GW), 0, N - 1)
        pclT = np.empty((4, NB), hf)
        pclT[0:3] = pcl[b, idx].T.astype(hf)
        pclT[3] = 0.0
        delta0 = np.zeros((4, NB), hf)
        delta0[3, GW:GW + NP] = c_global[np.clip(np.arange(g0, g0 + NP), 0, N - 1)]
        isL, isR = ch == 0, ch == 3
        m = dict(shared)
        m["pclT"] = pclT
        m["delta0"] = delta0
        m["eLp"] = pair(Wo, Wo) if isL else zpair3
        m["eLs"] = pair(Wo, np.zeros_like(Wo)) if isL else zpair3
        m["eLnp"] = pair(-Wo, -Wo) if isL else zpair3
        m["eLns"] = pair(-Wo, np.zeros_like(Wo)) if isL else zpair3
        m["eRp"] = pair(Wo, Wo) if isR else zpair3
        m["eRs"] = pair(Wo, np.zeros_like(Wo)) if isR else zpair3
        m["eRnp"] = pair(-Wo, -Wo) if isR else zpair3
        m["eRns"] = pair(-Wo, np.zeros_like(Wo)) if isR else zpair3
        m["flagL"] = np.full((4, 1), 1.0 if isL else 0.0, np.float32)
        m["flagR"] = np.full((4, 1), 1.0 if isR else 0.0, np.float32)
        in_maps.append(m)
    return in_maps


_CACHED = {}


def _get_program(reps=1):
    if reps not in _CACHED:
        _CACHED[reps] = build_program(reps)
    return _CACHED[reps]


def kernel(**inputs):
    nc = _get_program(1)
    in_maps = host_prep(inputs)
    res = run_bass_kernel_spmd(nc, in_maps, list(range(N_CORES)))
    pcl = np.asarray(inputs["pcl_noisy"], np.float32)
    out = np.empty((B, N, D), np.float32)
    for core in range(N_CORES):
        b, ch = core // 4, core % 4
        sl = slice(ch * CHUNK, (ch + 1) * CHUNK)
        out[b, sl] = pcl[b, sl] + res.results[core]["outT"][0:3].T.astype(np.float32)
    return out


# revision 3
# speedup vs baseline: 1.4748x; 1.4748x over previous
"""Trainium2 Bass kernel for nn_DenoiseNet (langevin point-cloud denoiser).

Strategy (8 NeuronCores, SPMD, zero inter-core communication):
  - Shard over B(2) x 4 contiguous N-chunks of 4096 points, each core padded
    with a 64-point halo on both sides (dependency cone grows 3 pts/step,
    4 steps -> 12 needed). Global-edge clipping handled exactly via per-core
    weight data (zeros on interior cores), so one program runs on all cores.
  - Feature-major fp16 layout [128 feat, (k, n) cols]. Sliding-window gather
    and scatter_add become free-dim shifted access patterns; the scatter
    k-sum and the delta update ride matmul PSUM accumulation.
  - First score-net layer split: h0 = relu(W0g.T delta[n+off_k] + Gk[n]),
    with Gk = feat@W0[3:] + b0 + W0g.T(pcl_noisy[n+off_k] - pcl_noisy[n])
    precomputed once on device. Tracking delta (= pcl - pcl_noisy) keeps
    fp16 rounding off the large pcl values.
  - Each step runs as three software-pipelined passes (layer0 / block1 /
    block2+scatter) over 512-col tiles, sharing one 6-deep PSUM pool;
    elementwise ops are greedily load-balanced across ACT/DVE/GPSIMD.
"""

import sys
import numpy as np

for _p in ("/opt/trn_rl_repo",):
    if _p not in sys.path:
        sys.path.insert(0, _p)

import concourse.bass as bass
import concourse.bacc as bacc
import concourse.tile as tile
from concourse import mybir
from concourse.bass_utils import run_bass_kernel_spmd

# ---- problem constants (hardcoded per harness contract) ----
B, N, D = 2, 16384, 3
F = 128
K = 4
OFF = [-2, -1, 0, 1]
STEPS, S0, DECAY = 4, 0.2, 0.95
CHUNK, HALO, GW = 4096, 64, 2
NP = CHUNK + 2 * HALO          # 4224 local points
NB = NP + 2 * GW               # 4228 buffer cols (with guards)
R4 = K * NP                    # 16896 (k,n) columns
N_CORES = 8

f32 = mybir.dt.float32
f16 = mybir.dt.float16
AF = mybir.ActivationFunctionType
ALU = mybir.AluOpType

_CH512 = [(c * 512, min(512, NP - c * 512)) for c in range((NP + 511) // 512)]
_CHNB = [(c * 512, min(512, NB - c * 512)) for c in range((NB + 511) // 512)]


def build_program(reps=1, loop_n=0):
    """Build the SPMD Bass/Tile program. Returns compiled Bacc module."""
    nc = bacc.Bacc("TRN2", target_bir_lowering=False, debug=False)

    def inp(name, shape, dt):
        return nc.dram_tensor(name, list(shape), dt, kind="ExternalInput").ap()

    d_pclT = inp("pclT", (4, NB), f16)
    d_delta0 = inp("delta0", (4, NB), f16)
    d_Wf1 = inp("Wf1", (3, F), f16)
    d_bf1 = inp("bf1", (F, 1), f32)
    d_WfW = inp("WfW", (F, F), f16)
    d_bg = inp("bg", (F, 1), f32)
    d_W0g = inp("W0g", (3, F), f16)
    d_W0gn = inp("W0gn", (3, F), f16)
    d_I128 = inp("I128", (F, F), f16)
    d_Wb1 = inp("Wb1", (F, F), f16)
    d_Wb2 = inp("Wb2", (F, F), f16)
    d_bb1 = inp("bb1", (F, 1), f32)
    d_bb2 = inp("bb2", (F, 1), f32)
    d_WoS = inp("WoS", (F, 3 * STEPS), f16)
    d_I4 = inp("I4aug", (4, 3 * STEPS), f16)
    d_eL = inp("eL", (F, 3 * STEPS), f16)
    d_eLn = inp("eLn", (F, 3 * STEPS), f16)
    d_eR = inp("eR", (F, 3 * STEPS), f16)
    d_eRn = inp("eRn", (F, 3 * STEPS), f16)
    d_flagL = inp("flagL", (4, 1), f32)
    d_flagR = inp("flagR", (4, 1), f32)
    d_out = nc.dram_tensor("outT", [4, CHUNK], f16, kind="ExternalOutput").ap()

    from contextlib import ExitStack
    with tile.TileContext(nc) as tc, ExitStack() as ctx:
        cpool = ctx.enter_context(tc.tile_pool(name="const", bufs=1))
        hpool = ctx.enter_context(tc.tile_pool(name="h", bufs=4))
        tpool = ctx.enter_context(tc.tile_pool(name="tiny", bufs=2))
        psp = ctx.enter_context(tc.tile_pool(name="ps", bufs=6, space="PSUM"))
        pspp = ctx.enter_context(tc.tile_pool(name="psP", bufs=2, space="PSUM"))
        h0pool = ctx.enter_context(tc.tile_pool(name="h0p", bufs=36))

        def load(dram, shape, dt, tag):
            t = cpool.tile(list(shape), dt, tag=tag)
            nc.sync.dma_start(t[:], dram[:])
            return t

        pclT = load(d_pclT, (4, NB), f16, "pclT")
        delta_a = load(d_delta0, (4, NB), f16, "delta_a")
        delta_b = load(d_delta0, (4, NB), f16, "delta_b")
        Wf1 = load(d_Wf1, (3, F), f16, "Wf1")
        bf1 = load(d_bf1, (F, 1), f32, "bf1")
        WfW = load(d_WfW, (F, F), f16, "WfW")
        bg = load(d_bg, (F, 1), f32, "bg")
        W0g = load(d_W0g, (3, F), f16, "W0g")
        W0gn = load(d_W0gn, (3, F), f16, "W0gn")
        I128 = load(d_I128, (F, F), f16, "I128")
        Wb1 = load(d_Wb1, (F, F), f16, "Wb1")
        Wb2 = load(d_Wb2, (F, F), f16, "Wb2")
        bb1 = load(d_bb1, (F, 1), f32, "bb1")
        bb2 = load(d_bb2, (F, 1), f32, "bb2")
        WoS = load(d_WoS, (F, 3 * STEPS), f16, "WoS")
        I4 = load(d_I4, (4, 3 * STEPS), f16, "I4")
        eL = load(d_eL, (F, 3 * STEPS), f16, "eL")
        eLn = load(d_eLn, (F, 3 * STEPS), f16, "eLn")
        eR = load(d_eR, (F, 3 * STEPS), f16, "eR")
        eRn = load(d_eRn, (F, 3 * STEPS), f16, "eRn")
        flagL = load(d_flagL, (4, 1), f32, "flagL")
        flagR = load(d_flagR, (4, 1), f32, "flagR")

        Gk = cpool.tile([F, R4], f16, tag="Gk")
        h2_a = cpool.tile([F, R4], f16, tag="h2_a")
        h2_b = cpool.tile([F, R4], f16, tag="h2_b")
        A0e = cpool.tile([F, NB], f16, tag="A0e")
        G0 = cpool.tile([F, NP], f16, tag="G0")

        # greedy engine balancer for elementwise work
        load_ns = {"ACT": 0.0, "DVE": 0.0, "GP": 0.0}

        def pick(cands):
            eng, cost, fn = min(cands, key=lambda c: load_ns[c[0]] + c[1])
            load_ns[eng] += cost
            fn()

        def relu_op(dst, src, fd, bias=None):
            # psum -> sbuf relu, optional per-partition bias
            def on_act():
                nc.scalar.activation(dst, src, AF.Relu,
                                     bias=(bias[:, :] if bias is not None else 0.0))
            def on_dve():
                if bias is not None:
                    nc.vector.tensor_scalar(dst, src, bias[:, :], 0.0, ALU.add, ALU.max)
                else:
                    nc.vector.tensor_scalar_max(dst, src, 0.0)
            pick([("ACT", (fd + 212) * 0.833 + 16, on_act),
                  ("DVE", (fd + 60) * 1.042 + 15, on_dve)])

        def copy_op(dst, src, fd):
            def on_act():
                nc.scalar.activation(dst, src, AF.Copy)
            def on_dve():
                nc.vector.tensor_copy(dst, src)
            pick([("ACT", (fd + 212) * 0.833 + 16, on_act),
                  ("DVE", (fd + 60) * 1.042 + 15, on_dve)])

        def add_op(dst, a, b, fd):
            def on_dve():
                nc.vector.tensor_add(dst, a, b)
            def on_gp():
                nc.gpsimd.tensor_add(dst, a, b)
            pick([("DVE", (fd / 2 + 52) * 1.042 + 15, on_dve),
                  ("GP", fd * 2.3, on_gp)])

        # one column at the k=2/k=3 boundary is read (as cone garbage) by the
        # interleaved scatter before any tile writes it on step 0
        nc.vector.memset(h2_a[:, 3 * NP - 1:3 * NP], 0.0)
        nc.vector.memset(h2_b[:, 3 * NP - 1:3 * NP], 0.0)

        # ---------------- preamble: A0e, G0, Gk ----------------
        for ci, (c0, fd) in enumerate(_CHNB):
            ps = psp.tile([F, 512], f32, tag="ps")
            nc.tensor.matmul(ps[:, :fd], W0g[:, :], pclT[0:3, c0:c0 + fd],
                             start=True, stop=True)
            copy_op(A0e[:, c0:c0 + fd], ps[:, :fd], fd)
        for ci, (c0, fd) in enumerate(_CH512):
            ps = psp.tile([F, 512], f32, tag="ps")
            nc.tensor.matmul(ps[:, :fd], Wf1[:, :], pclT[0:3, GW + c0:GW + c0 + fd],
                             start=True, stop=True)
            hf = hpool.tile([F, 1024], f16, tag="h0")
            nc.scalar.activation(hf[:, :fd], ps[:, :fd], AF.Relu, bias=bf1[:, :])
            ps2 = psp.tile([F, 512], f32, tag="ps")
            nc.tensor.matmul(ps2[:, :fd], WfW[:, :], hf[:, :fd], start=True, stop=False)
            nc.tensor.matmul(ps2[:, :fd], W0gn[:, :], pclT[0:3, GW + c0:GW + c0 + fd],
                             start=False, stop=True)
            nc.scalar.activation(G0[:, c0:c0 + fd], ps2[:, :fd], AF.Identity, bias=bg[:, :])
        for k in range(K):
            for c0, fd in _CH512:
                add_op(Gk[:, k * NP + c0:k * NP + c0 + fd], G0[:, c0:c0 + fd],
                       A0e[:, GW + OFF[k] + c0:GW + OFF[k] + c0 + fd], fd)

        # ---------------- langevin steps ----------------
        def emit_rep(final_rep):
            for step in range(STEPS):
                d_in = delta_a if step % 2 == 0 else delta_b
                d_out_t = delta_b if step % 2 == 0 else delta_a
                h2 = h2_a if step % 2 == 0 else h2_b
                final = (step == STEPS - 1) and final_rep
                s3 = slice(3 * step, 3 * step + 3)

                def emit_passA(cb):
                    c0, fd = _CH512[cb]
                    for k in range(K):
                        hcol = k * NP + c0
                        ps = psp.tile([F, 512], f32, tag="ps")
                        nc.tensor.matmul(
                            ps[:, :fd], W0g[:, :],
                            d_in[0:3, GW + OFF[k] + c0:GW + OFF[k] + c0 + fd],
                            start=True, stop=False)
                        nc.tensor.matmul(ps[:, :fd], I128[:, :],
                                         Gk[:, hcol:hcol + fd], start=False, stop=True)
                        h0 = h0pool.tile([F, 512], f16, tag="h0")
                        relu_op(h0[:, :fd], ps[:, :fd], fd)
                        h0s[(k, cb)] = h0

                def emit_passB(cb):
                    c0, fd = _CH512[cb]
                    for k in range(K):
                        hcol = k * NP + c0
                        h0 = h0s[(k, cb)]
                        ps = psp.tile([F, 512], f32, tag="ps")
                        nc.tensor.matmul(ps[:, :fd], Wb1[:, :], h0[:, :fd],
                                         start=True, stop=True)
                        r1 = hpool.tile([F, 512], f16, tag="r1")
                        relu_op(r1[:, :fd], ps[:, :fd], fd, bias=bb1)
                        add_op(h2[:, hcol:hcol + fd], h0[:, :fd], r1[:, :fd], fd)

                def emit_passC(cb):
                    c0, fd = _CH512[cb]
                    for k in range(K):
                        hcol = k * NP + c0
                        ps = psp.tile([F, 512], f32, tag="ps")
                        nc.tensor.matmul(ps[:, :fd], Wb2[:, :],
                                         h2[:, hcol:hcol + fd], start=True, stop=True)
                        r2 = hpool.tile([F, 512], f16, tag="r2")
                        relu_op(r2[:, :fd], ps[:, :fd], fd, bias=bb2)
                        add_op(h2[:, hcol:hcol + fd], h2[:, hcol:hcol + fd],
                               r2[:, :fd], fd)

                def mirror_fix(flag, src_l, dst_ls):
                    # mirror guards at global edges (flag=0 -> no-op on interior)
                    for dst_l in dst_ls:
                        t = tpool.tile([4, 1], f16, tag="mir")
                        nc.vector.tensor_sub(t[0:3, :],
                                             d_out_t[0:3, GW + src_l:GW + src_l + 1],
                                             d_out_t[0:3, GW + dst_l:GW + dst_l + 1])
                        nc.vector.tensor_scalar_mul(t[0:3, :], t[0:3, :], flag[0:3, :])
                        nc.vector.tensor_add(d_out_t[0:3, GW + dst_l:GW + dst_l + 1],
                                             d_out_t[0:3, GW + dst_l:GW + dst_l + 1],
                                             t[0:3, :])

                def emit_scatter(cb):
                    c0, fd = _CH512[cb]
                    ps = pspp.tile([4, 512], f32, tag="psP")
                    for k in range(K):
                        st = k * NP + c0 - OFF[k]
                        nc.tensor.matmul(ps[0:3, :fd], WoS[:, s3],
                                         h2[:, st:st + fd],
                                         start=(k == 0), stop=False)
                    if cb == 0:
                        pcol = ps[0:3, HALO:HALO + 1]
                        for col in (HALO, HALO + 1, NP + HALO):
                            nc.tensor.matmul(pcol, eL[:, s3], h2[:, col:col + 1],
                                             start=False, stop=False)
                        nc.tensor.matmul(pcol, eLn[:, s3],
                                         h2[:, 3 * NP + HALO - 1:3 * NP + HALO],
                                         start=False, stop=False)
                    if cb == len(_CH512) - 1:
                        lN = HALO + CHUNK - 1
                        pN = ps[0:3, lN - c0:lN - c0 + 1]
                        nc.tensor.matmul(pN, eR[:, s3], h2[:, 3 * NP + lN:3 * NP + lN + 1],
                                         start=False, stop=False)
                        for col in (lN + 2, NP + lN + 1):
                            nc.tensor.matmul(pN, eRn[:, s3], h2[:, col:col + 1],
                                             start=False, stop=False)
                        nc.tensor.matmul(ps[0:3, lN - 1 - c0:lN - c0], eRn[:, s3],
                                         h2[:, lN + 1:lN + 2], start=False, stop=False)
                    nc.tensor.matmul(ps[0:3, :fd], I4[:, s3],
                                     d_in[0:4, GW + c0:GW + c0 + fd],
                                     start=False, stop=True)
                    nc.vector.tensor_copy(d_out_t[0:3, GW + c0:GW + c0 + fd], ps[0:3, :fd])
                    load_ns["DVE"] += (fd + 60) * 1.042 + 15

                h0s = {}
                nblk = len(_CH512)
                for cb in range(nblk + 5):
                    if cb < nblk:
                        emit_passA(cb)
                    if 0 <= cb - 2 < nblk:
                        emit_passB(cb - 2)
                    if 0 <= cb - 3 < nblk:
                        emit_passC(cb - 3)
                    if 0 <= cb - 5 < nblk:
                        emit_scatter(cb - 5)

                if final:
                    nc.sync.dma_start(
                        d_out[:, :], d_out_t[0:4, GW + HALO:GW + HALO + CHUNK])
                else:
                    mirror_fix(flagL, HALO, (HALO - 2, HALO - 1))
                    mirror_fix(flagR, HALO + CHUNK - 1, (HALO + CHUNK,))

        if loop_n:
            with tc.For_i(0, loop_n, 1):
                emit_rep(False)
            emit_rep(True)
        else:
            for rep in range(reps):
                emit_rep(rep == reps - 1)

    nc.compile()
    return nc


def host_prep(inputs):
    """Slice/transpose/pad inputs per core; build weight-variant constants."""
    pcl = np.asarray(inputs["pcl_noisy"], np.float32)
    Wf1 = np.asarray(inputs["Wf1"], np.float32)
    bf1 = np.asarray(inputs["bf1"], np.float32)
    Wf2 = np.asarray(inputs["Wf2"], np.float32)
    bf2 = np.asarray(inputs["bf2"], np.float32)
    W0 = np.asarray(inputs["W0"], np.float32)
    b0 = np.asarray(inputs["b0"], np.float32)
    Wb = np.asarray(inputs["Wb"], np.float32)
    bb = np.asarray(inputs["bb"], np.float32)
    Wo = np.asarray(inputs["Wo"], np.float32)
    bo = np.asarray(inputs["bo"], np.float32)

    W0g = W0[:3]
    WfW = Wf2 @ W0[3:]
    bg = bf2 @ W0[3:] + b0
    offs = np.arange(-(K - 1) // 2, (K - 1) // 2 + 1)
    nbr = np.clip(np.arange(N)[:, None] + offs, 0, N - 1).reshape(-1)
    c_global = np.bincount(nbr, minlength=N).astype(np.float32)

    svals = [S0 * DECAY ** i for i in range(STEPS)]
    WoS = np.concatenate([s * Wo for s in svals], axis=1)          # [128, 12]
    I4 = np.zeros((4, 3 * STEPS), np.float32)
    for i, s in enumerate(svals):
        blk = np.eye(4, 3, dtype=np.float32)
        blk[3, 0:3] = s * bo
        I4[:, 3 * i:3 * i + 3] = blk

    hf = np.float16
    shared = {
        "Wf1": Wf1.astype(hf), "bf1": bf1.reshape(F, 1),
        "WfW": WfW.astype(hf), "bg": bg.reshape(F, 1),
        "W0g": W0g.astype(hf), "W0gn": (-W0g).astype(hf),
        "I128": np.eye(F, dtype=np.float32).astype(hf),
        "Wb1": Wb[0].astype(hf), "Wb2": Wb[1].astype(hf),
        "bb1": bb[0].reshape(F, 1), "bb2": bb[1].reshape(F, 1),
        "WoS": WoS.astype(hf),
        "I4aug": I4.astype(hf),
    }
    zeros_e = np.zeros((F, 3 * STEPS), np.float16)
    in_maps = []
    for core in range(N_CORES):
        b, ch = core // 4, core % 4
        g0 = ch * CHUNK - HALO
        idx = np.clip(np.arange(g0 - GW, g0 + NP + GW), 0, N - 1)
        pclT = np.empty((4, NB), np.float16)
        pclT[0:3] = pcl[b, idx].T.astype(np.float16)
        pclT[3] = 0.0
        delta0 = np.zeros((4, NB), np.float16)
        delta0[3, GW:GW + NP] = c_global[np.clip(np.arange(g0, g0 + NP), 0, N - 1)]
        isL, isR = ch == 0, ch == 3
        m = dict(shared)
        m["pclT"] = pclT
        m["delta0"] = delta0
        m["eL"] = (WoS.astype(hf) if isL else zeros_e)
        m["eLn"] = ((-WoS).astype(hf) if isL else zeros_e)
        m["eR"] = (WoS.astype(hf) if isR else zeros_e)
        m["eRn"] = ((-WoS).astype(hf) if isR else zeros_e)
        m["flagL"] = np.full((4, 1), 1.0 if isL else 0.0, np.float32)
        m["flagR"] = np.full((4, 1), 1.0 if isR else 0.0, np.float32)
        in_maps.append(m)
    return in_maps


_CACHED = {}


def _get_program(reps=1):
    if reps not in _CACHED:
        _CACHED[reps] = build_program(reps)
    return _CACHED[reps]


def kernel(**inputs):
    nc = _get_program(1)
    in_maps = host_prep(inputs)
    res = run_bass_kernel_spmd(nc, in_maps, list(range(N_CORES)))
    pcl = np.asarray(inputs["pcl_noisy"], np.float32)
    out = np.empty((B, N, D), np.float32)
    for core in range(N_CORES):
        b, ch = core // 4, core % 4
        sl = slice(ch * CHUNK, (ch + 1) * CHUNK)
        out[b, sl] = pcl[b, sl] + res.results[core]["outT"][0:3].T.astype(np.float32)
    return out

